# revision 3
# baseline (speedup 1.0000x reference)
"""Trainium2 Bass kernel: single-channel 2D conv (valid), X[8192,8192] * w[5,5] + bias.

Strategy: row-shard X across 8 NeuronCores with a (kh-1)-row halo (host-side
overlapping slices; weight/bias replicated). On each core, the conv is computed
as 5 PSUM-accumulated TensorE matmuls per output tile: for each kernel column
dj, a banded stationary matrix A_dj[k, m] = w[k-m, dj] (0 <= k-m < 5) contracts
over up to 128 input rows to produce up to 124 output rows of the
column-direction conv, while the moving operand is the input tile shifted by dj
columns. Accumulating the 5 dj-shifts in PSUM yields the full 5x5 conv.

All data is bf16 (operands and HBM traffic): the PE streams bf16 at 1
col/cycle @ 2.4 GHz (vs ~1.7 cycles/col for fp32r), LDWEIGHTS gets fast-weight
-load (stationary padded to 128 cols), and HBM bytes halve in both directions.
PSUM accumulation stays fp32; the output is stored bf16 and upconverted to
fp32 on the host. End-to-end rel error ~3e-3 (vs 2e-2 budget).
"""

import numpy as np
import ml_dtypes

import concourse.bass as bass
import concourse.mybir as mybir
from concourse import bacc
from concourse import bass_utils
from concourse.tile import TileContext

H = 8192
W = 8192
KH = 5
KW = 5
OH = H - KH + 1  # 8188
OW = W - KW + 1  # 8188

NCORES = 8
ROWS_OUT = 1024  # output rows per core (8*1024 = 8192 >= 8188; tail cropped)
ROWS_IN = ROWS_OUT + KH - 1  # 1028

BAND_OUT = 124  # output rows per matmul band (K=128 partitions -> M=124)
AW = 128  # stationary width per dj slice (padded to 128 cols for FWL)
SUB_W = 512  # matmul moving free dim (one PSUM bank of fp32)

# 8 full bands of 124 output rows + a 32-row tail band
_BANDS = [(124 * i, 124) for i in range(8)] + [(992, 32)]
# 16 uniform column subtiles; the last one overlaps
_SUB_STARTS = [512 * i for i in range(15)] + [OW - SUB_W]

_PROGRAM_CACHE = {}

# Populated by the most recent kernel() call when tracing is enabled via
# TRACE=1 (module attr) — used by test.py for HW exec time reporting.
TRACE = False
LAST_RUN = {}


def _build_program(bias_val: float):
    f32 = mybir.dt.float32
    bf16 = mybir.dt.bfloat16

    nc = bacc.Bacc("TRN2", target_bir_lowering=False, debug=False, num_devices=NCORES)

    Xs = nc.dram_tensor("Xs", [ROWS_IN, W], bf16, kind="ExternalInput")
    Aw = nc.dram_tensor("Aw", [128, KW * AW], bf16, kind="ExternalInput")
    # Output rows padded to 8192 cols so every store row is a full-line HBM
    # write; host crops to 8188.
    Y = nc.dram_tensor("Y", [ROWS_OUT, W], bf16, kind="ExternalOutput")

    with TileContext(nc) as tc:
        with (
            tc.tile_pool(name="const", bufs=1) as cpool,
            tc.tile_pool(name="inp", bufs=3) as in_pool,
            tc.tile_pool(name="outp", bufs=3) as out_pool,
            tc.tile_pool(name="psum", bufs=8, space="PSUM") as psum_pool,
        ):
            A_t = cpool.tile([128, KW * AW], bf16)
            nc.sync.dma_start(A_t[:], Aw.ap())

            # Loads ride the gpsimd SWDGE queue alone (spread over all 16
            # SDMA engines, never blocked behind compute-dependent stores);
            # stores alternate across the two HWDGE rings (sync / scalar).
            qs = [nc.sync, nc.scalar]
            si = 0
            for bi, (r0, rows_out) in enumerate(_BANDS):
                rows_in = rows_out + KH - 1
                in_t = in_pool.tile([rows_in, W], bf16)
                nc.gpsimd.dma_start(in_t[:], Xs.ap()[r0 : r0 + rows_in, :])
                out_t = out_pool.tile([rows_out, W], bf16)
                for ci, c0 in enumerate(_SUB_STARTS):
                    ps = psum_pool.tile([128, SUB_W], f32)
                    for dj in range(KW):
                        nc.tensor.matmul(
                            ps[:],
                            A_t[0:rows_in, dj * AW : dj * AW + AW],
                            in_t[:, c0 + dj : c0 + dj + SUB_W],
                            start=(dj == 0),
                            stop=(dj == KW - 1),
                        )
                    dst = out_t[0:rows_out, c0 : c0 + SUB_W]
                    # Alternate PSUM evacuation between DVE and ACT so
                    # neither engine becomes the bottleneck.
                    if bias_val == 0.0 and ci % 2 == 0:
                        nc.vector.tensor_copy(dst, ps[0:rows_out, :])
                    else:
                        nc.scalar.activation(
                            dst,
                            ps[0:rows_out, :],
                            mybir.ActivationFunctionType.Copy,
                            bias=bias_val,
                        )
                # Store in 4 chunks alternating across the two HWDGE rings.
                n_chunks = 4 if rows_out == BAND_OUT else 2
                bounds = [rows_out * i // n_chunks for i in range(n_chunks + 1)]
                for ci in range(n_chunks):
                    lo, hi = bounds[ci], bounds[ci + 1]
                    qs[si % 2].dma_start(Y.ap()[r0 + lo : r0 + hi, :], out_t[lo:hi, :])
                    si += 1

    nc.compile()
    return nc


def kernel(X, weight, bias):
    X = np.ascontiguousarray(np.asarray(X, dtype=np.float32))
    weight = np.asarray(weight, dtype=np.float32)
    bias = np.asarray(bias, dtype=np.float32)
    assert X.shape == (H, W) and weight.shape == (KH, KW)

    bias_val = float(bias.reshape(-1)[0])
    key = bias_val
    nc = _PROGRAM_CACHE.get(key)
    if nc is None:
        nc = _build_program(bias_val)
        _PROGRAM_CACHE[key] = nc

    # Banded stationary matrices: A[k, dj*128 + m] = w[k-m, dj] for 0<=k-m<5
    A = np.zeros((128, KW * AW), dtype=np.float32)
    m = np.arange(BAND_OUT)
    for dj in range(KW):
        for di in range(KH):
            A[m + di, dj * AW + m] = weight[di, dj]
    A = A.astype(ml_dtypes.bfloat16)

    # Row-shard with halo; pad the bottom so every core gets ROWS_IN rows.
    Xp = np.zeros((NCORES * ROWS_OUT + KH - 1, W), dtype=ml_dtypes.bfloat16)
    Xp[:H] = X.astype(ml_dtypes.bfloat16)
    in_maps = [
        {"Xs": Xp[c * ROWS_OUT : c * ROWS_OUT + ROWS_IN], "Aw": A}
        for c in range(NCORES)
    ]

    res = bass_utils.run_bass_kernel_spmd(
        nc, in_maps, core_ids=list(range(NCORES)), trace=TRACE
    )
    LAST_RUN.clear()
    LAST_RUN.update(
        exec_time_ns=res.exec_time_ns,
        instructions_and_trace=res.instructions_and_trace,
        profile_json=res.profile_json,
    )

    out = np.concatenate([res.results[c]["Y"] for c in range(NCORES)], axis=0)
    return np.ascontiguousarray(out[:OH, :OW].astype(np.float32))


# revision 5
# speedup vs baseline: 3.4245x; 3.4245x over previous
"""Trainium2 Bass kernel: single-channel 2D conv (valid), X[8192,8192] * w[5,5] + bias.

Strategy: row-shard X across 8 NeuronCores with a (kh-1)-row halo (host-side
overlapping slices; weight/bias replicated). On each core, the conv is computed
as 5 PSUM-accumulated TensorE matmuls per output tile: for each kernel column
dj, a banded stationary matrix A_dj[k, m] = w[k-m, dj] (0 <= k-m < 5) contracts
over up to 128 input rows to produce up to 124 output rows of the
column-direction conv, while the moving operand is the input tile shifted by dj
columns. Accumulating the 5 dj-shifts in PSUM yields the full 5x5 conv.

All data is bf16 (operands and HBM traffic): the PE streams bf16 at 1
col/cycle @ 2.4 GHz (vs ~1.7 cycles/col for fp32r), LDWEIGHTS gets fast-weight
-load (stationary padded to 128 cols), and HBM bytes halve in both directions.
PSUM accumulation stays fp32; the output is stored bf16 and upconverted to
fp32 on the host. End-to-end rel error ~3e-3 (vs 2e-2 budget).
"""

import numpy as np
import ml_dtypes

import concourse.bass as bass
import concourse.mybir as mybir
from concourse import bacc
from concourse import bass_utils
from concourse.tile import TileContext

H = 8192
W = 8192
KH = 5
KW = 5
OH = H - KH + 1  # 8188
OW = W - KW + 1  # 8188

NCORES = 8
ROWS_OUT = 1024  # output rows per core (8*1024 = 8192 >= 8188; tail cropped)
ROWS_IN = ROWS_OUT + KH - 1  # 1028

BAND_OUT = 124  # output rows per matmul band (K=128 partitions -> M=124)
AW = 128  # stationary width per dj slice (padded to 128 cols for FWL)
SUB_W = 512  # matmul moving free dim (one PSUM bank of fp32)

# 8 full bands of 124 output rows + a 32-row tail band
_BANDS = [(124 * i, 124) for i in range(8)] + [(992, 32)]
# 16 uniform column subtiles; the last one overlaps
_SUB_STARTS = [512 * i for i in range(15)] + [OW - SUB_W]

_PROGRAM_CACHE = {}

# Populated by the most recent kernel() call when tracing is enabled via
# TRACE=1 (module attr) — used by test.py for HW exec time reporting.
TRACE = False
LAST_RUN = {}


def _build_program(bias_val: float):
    f32 = mybir.dt.float32
    bf16 = mybir.dt.bfloat16

    nc = bacc.Bacc("TRN2", target_bir_lowering=False, debug=False, num_devices=NCORES)

    Xs = nc.dram_tensor("Xs", [ROWS_IN, W], bf16, kind="ExternalInput")
    Aw = nc.dram_tensor("Aw", [128, KW * AW], bf16, kind="ExternalInput")
    # Output rows padded to 8192 cols so every store row is a full-line HBM
    # write; host crops to 8188.
    Y = nc.dram_tensor("Y", [ROWS_OUT, W], bf16, kind="ExternalOutput")

    with TileContext(nc) as tc:
        with (
            tc.tile_pool(name="const", bufs=1) as cpool,
            tc.tile_pool(name="inp", bufs=4) as in_pool,
            tc.tile_pool(name="outp", bufs=3) as out_pool,
            tc.tile_pool(name="psum", bufs=8, space="PSUM") as psum_pool,
        ):
            A_t = cpool.tile([128, KW * AW], bf16)
            nc.sync.dma_start(A_t[:], Aw.ap())

            # All HBM traffic rides the gpsimd SWDGE queue: SWDGE spreads
            # 16KiB bf16 rows across all 16 SDMA engines, while the HWDGE
            # rings serialize them onto a single engine (~25 GB/s). Stores
            # are deferred by one band so their semaphore waits never block
            # the next band's load issue.
            pending = []
            for bi, (r0, rows_out) in enumerate(_BANDS):
                rows_in = rows_out + KH - 1
                in_t = in_pool.tile([rows_in, W], bf16)
                nc.gpsimd.dma_start(in_t[:], Xs.ap()[r0 : r0 + rows_in, :])
                if pending:
                    r0s, rout, t = pending.pop(0)
                    half = rout // 2
                    nc.gpsimd.dma_start(Y.ap()[r0s : r0s + half, :], t[0:half, :])
                    nc.gpsimd.dma_start(Y.ap()[r0s + half : r0s + rout, :], t[half:rout, :])
                out_t = out_pool.tile([rows_out, W], bf16)
                for ci, c0 in enumerate(_SUB_STARTS):
                    ps = psum_pool.tile([128, SUB_W], f32)
                    for dj in range(KW):
                        nc.tensor.matmul(
                            ps[:],
                            A_t[0:rows_in, dj * AW : dj * AW + AW],
                            in_t[:, c0 + dj : c0 + dj + SUB_W],
                            start=(dj == 0),
                            stop=(dj == KW - 1),
                        )
                    dst = out_t[0:rows_out, c0 : c0 + SUB_W]
                    # Alternate PSUM evacuation between DVE and ACT so
                    # neither engine becomes the bottleneck.
                    if bias_val == 0.0 and ci % 2 == 0:
                        nc.vector.tensor_copy(dst, ps[0:rows_out, :])
                    else:
                        nc.scalar.activation(
                            dst,
                            ps[0:rows_out, :],
                            mybir.ActivationFunctionType.Copy,
                            bias=bias_val,
                        )
                pending.append((r0, rows_out, out_t))
            while pending:
                r0s, rout, t = pending.pop(0)
                half = rout // 2
                nc.gpsimd.dma_start(Y.ap()[r0s : r0s + half, :], t[0:half, :])
                nc.gpsimd.dma_start(Y.ap()[r0s + half : r0s + rout, :], t[half:rout, :])

    nc.compile()
    return nc


def kernel(X, weight, bias):
    X = np.ascontiguousarray(np.asarray(X, dtype=np.float32))
    weight = np.asarray(weight, dtype=np.float32)
    bias = np.asarray(bias, dtype=np.float32)
    assert X.shape == (H, W) and weight.shape == (KH, KW)

    bias_val = float(bias.reshape(-1)[0])
    key = bias_val
    nc = _PROGRAM_CACHE.get(key)
    if nc is None:
        nc = _build_program(bias_val)
        _PROGRAM_CACHE[key] = nc

    # Banded stationary matrices: A[k, dj*128 + m] = w[k-m, dj] for 0<=k-m<5
    A = np.zeros((128, KW * AW), dtype=np.float32)
    m = np.arange(BAND_OUT)
    for dj in range(KW):
        for di in range(KH):
            A[m + di, dj * AW + m] = weight[di, dj]
    A = A.astype(ml_dtypes.bfloat16)

    # Row-shard with halo; pad the bottom so every core gets ROWS_IN rows.
    Xp = np.zeros((NCORES * ROWS_OUT + KH - 1, W), dtype=ml_dtypes.bfloat16)
    Xp[:H] = X.astype(ml_dtypes.bfloat16)
    in_maps = [
        {"Xs": Xp[c * ROWS_OUT : c * ROWS_OUT + ROWS_IN], "Aw": A}
        for c in range(NCORES)
    ]

    res = bass_utils.run_bass_kernel_spmd(
        nc, in_maps, core_ids=list(range(NCORES)), trace=TRACE
    )
    LAST_RUN.clear()
    LAST_RUN.update(
        exec_time_ns=res.exec_time_ns,
        instructions_and_trace=res.instructions_and_trace,
        profile_json=res.profile_json,
    )

    out = np.concatenate([res.results[c]["Y"] for c in range(NCORES)], axis=0)
    return np.ascontiguousarray(out[:OH, :OW].astype(np.float32))
